# revision 1
# baseline (speedup 1.0000x reference)
"""TRN2 Bass kernel for a fused LSTM cell:

    gates = [x, h] @ [Wf|Wi|Wc|Wo] + b
    c_t = sigmoid(f)*c_prev + sigmoid(i)*tanh(c~)
    h_t = sigmoid(o)*tanh(c_t)

Distribution: data-parallel over the batch (4096 rows -> 512 per core x 8
cores), no collectives; the host shards inputs and gathers outputs.

Per-core layout is transposed (hidden on partitions, batch on the free
dim): stationary operand = W tile [128k, 128col], moving operand =
combined^T [128k, 512 batch].  Matmuls run in float32r (full PE rate at
moving dim >= 256, ~1e-4 relative error).  The gate bias is folded into
the ScalarEngine activation (bias varies along partitions in this
layout, which is exactly what the ACT bias operand broadcasts).
"""

import numpy as np
from contextlib import ExitStack

import concourse.bass as bass
import concourse.tile as tile
from concourse import bacc, mybir
from concourse.bass_utils import run_bass_kernel_spmd

B = 4096          # batch
D_IN = 2048       # input size
D_HID = 2048      # hidden size
K = D_IN + D_HID  # contraction dim = 4096
NCORES = 8
BC = B // NCORES  # batch per core = 512
KT = K // 128     # 32 k-tiles
HT = D_HID // 128 # 16 hidden tiles
F32 = mybir.dt.float32
F32R = mybir.dt.float32r
SIG = mybir.ActivationFunctionType.Sigmoid
TANH = mybir.ActivationFunctionType.Tanh

_CACHE = {}


def _build(reps: int = 1):
    """Build + compile the per-core program (identical on all cores)."""
    nc = bacc.Bacc("TRN2", target_bir_lowering=False, debug=False)

    w = nc.declare_dram_parameter("w", [4, HT, KT, 128, 128], F32R, isOutput=False)
    combt = nc.declare_dram_parameter("combt", [KT, 128, BC], F32R, isOutput=False)
    cprevt = nc.declare_dram_parameter("cprevt", [HT, 128, BC], F32, isOutput=False)
    bias = nc.declare_dram_parameter("bias", [128, 4 * HT], F32, isOutput=False)
    h_out = nc.declare_dram_parameter("h_out", [HT, 128, BC], F32, isOutput=True)
    c_out = nc.declare_dram_parameter("c_out", [HT, 128, BC], F32, isOutput=True)

    with ExitStack() as ctx:
        tc = ctx.enter_context(tile.TileContext(nc))
        res = ctx.enter_context(tc.tile_pool(name="res", bufs=1))
        wpool = ctx.enter_context(tc.tile_pool(name="wpool", bufs=3))
        ps = ctx.enter_context(tc.tile_pool(name="ps", bufs=2, space="PSUM"))
        ep = ctx.enter_context(tc.tile_pool(name="ep", bufs=3))

        comb_sb = res.tile([128, KT, BC], F32R)
        nc.sync.dma_start(out=comb_sb, in_=combt.rearrange("n p m -> p n m"))
        cprev_sb = res.tile([128, HT, BC], F32)
        nc.sync.dma_start(out=cprev_sb, in_=cprevt.rearrange("n p m -> p n m"))
        bias_sb = res.tile([128, 4 * HT], F32)
        nc.sync.dma_start(out=bias_sb, in_=bias[:, :])

        for _ in range(reps):
            for ht in range(HT):
                gate_sb = []
                for g in range(4):
                    wg = wpool.tile([128, KT, 128], F32R, tag="wg")
                    nc.sync.dma_start(
                        out=wg, in_=w[g, ht].rearrange("n p m -> p n m")
                    )
                    acc = ps.tile([128, BC], F32, tag=f"acc{g}")
                    for k in range(KT):
                        nc.tensor.matmul(
                            acc, lhsT=wg[:, k, :], rhs=comb_sb[:, k, :],
                            start=(k == 0), stop=(k == KT - 1),
                        )
                    out_g = ep.tile([128, BC], F32, tag=f"gate{g}")
                    nc.scalar.activation(
                        out_g, acc, TANH if g == 2 else SIG,
                        bias=bias_sb[:, g * HT + ht:g * HT + ht + 1],
                    )
                    gate_sb.append(out_g)
                f_s, i_s, ch, o_s = gate_sb
                t1 = ep.tile([128, BC], F32, tag="t1")
                nc.vector.tensor_mul(t1, f_s, cprev_sb[:, ht, :])
                t2 = ep.tile([128, BC], F32, tag="t2")
                nc.vector.tensor_mul(t2, i_s, ch)
                ct = ep.tile([128, BC], F32, tag="ct")
                nc.vector.tensor_add(ct, t1, t2)
                tct = ep.tile([128, BC], F32, tag="tct")
                nc.scalar.activation(tct, ct, TANH)
                htl = ep.tile([128, BC], F32, tag="htl")
                nc.vector.tensor_mul(htl, o_s, tct)
                nc.sync.dma_start(out=c_out[ht], in_=ct)
                nc.sync.dma_start(out=h_out[ht], in_=htl)
    nc.compile()
    return nc


def _get_nc(reps: int = 1):
    if reps not in _CACHE:
        _CACHE[reps] = _build(reps)
    return _CACHE[reps]


def _prep_inputs(x_t, h_prev, c_prev, Wf, bf, Wi, bi, Wc, bc, Wo, bo):
    # W packed as [4, HT, KT, 128, 128] so each (gate, ht) strip is one
    # contiguous 2 MiB DMA.
    w_pack = np.empty((4, HT, KT, 128, 128), np.float32)
    for g, W in enumerate((Wf, Wi, Wc, Wo)):
        # W [K, D_HID] -> [KT, 128, HT, 128] -> [HT, KT, 128, 128]
        w_pack[g] = np.asarray(W).reshape(KT, 128, HT, 128).transpose(2, 0, 1, 3)

    # bias packed [128, 4*HT]; column g*HT+ht holds b_g[ht*128:(ht+1)*128]
    b_pack = np.stack([np.asarray(b).reshape(HT, 128) for b in (bf, bi, bc, bo)])
    b_pack = b_pack.transpose(2, 0, 1).reshape(128, 4 * HT).copy()

    combT = np.concatenate([np.asarray(x_t), np.asarray(h_prev)], axis=1).T
    cprevT = np.asarray(c_prev).T

    in_maps = []
    for c in range(NCORES):
        sl = slice(c * BC, (c + 1) * BC)
        in_maps.append({
            "w": w_pack,
            "bias": b_pack,
            "combt": np.ascontiguousarray(combT[:, sl]).reshape(KT, 128, BC),
            "cprevt": np.ascontiguousarray(cprevT[:, sl]).reshape(HT, 128, BC),
        })
    return in_maps


def kernel(x_t, h_prev, c_prev, Wf, bf, Wi, bi, Wc, bc, Wo, bo, _reps=1):
    nc = _get_nc(_reps)
    in_maps = _prep_inputs(x_t, h_prev, c_prev, Wf, bf, Wi, bi, Wc, bc, Wo, bo)
    r = run_bass_kernel_spmd(nc, in_maps, core_ids=list(range(NCORES)))
    h_t = np.empty((D_HID, B), np.float32)
    c_t = np.empty((D_HID, B), np.float32)
    for c in range(NCORES):
        sl = slice(c * BC, (c + 1) * BC)
        h_t[:, sl] = r.results[c]["h_out"].reshape(D_HID, BC)
        c_t[:, sl] = r.results[c]["c_out"].reshape(D_HID, BC)
    return (np.ascontiguousarray(h_t.T), np.ascontiguousarray(c_t.T))


# revision 4
# speedup vs baseline: 1.6831x; 1.6831x over previous
"""TRN2 Bass kernel for a fused LSTM cell:

    gates = [x, h] @ [Wf|Wi|Wc|Wo] + b
    c_t = sigmoid(f)*c_prev + sigmoid(i)*tanh(c~)
    h_t = sigmoid(o)*tanh(c_t)

Distribution: tensor-parallel over hidden units (2048 -> 256 per core x 8
cores), no collectives; the host shards W / c_prev / bias by hidden slice
and gathers the output slices.

Each core keeps its whole W slice [4096, 4x256] RESIDENT in SBUF (16 MiB,
loaded once) and streams the full-batch activations through it, so the
fused gate weight is read from HBM exactly once per chip (vs 8x for data
parallelism) - HBM traffic per iteration drops from ~1.2 GB to ~0.74 GB,
below the compute time, making the kernel compute-bound as intended.

Per-core layout is transposed (hidden on partitions, batch on the free
dim): stationary operand = W tile [128k, 128col], moving operand =
combined^T [128k, 512 batch].  Matmuls run in float32r (full PE rate at
moving dim >= 256, ~1e-4 relative error).  The gate bias is folded into
the ScalarEngine activation (bias varies along partitions in this
layout, which is exactly what the ACT bias operand broadcasts).
"""

import numpy as np
from contextlib import ExitStack

import concourse.bass as bass
import concourse.tile as tile
from concourse import bacc, mybir
from concourse.bass_utils import run_bass_kernel_spmd

B = 4096          # batch
D_IN = 2048       # input size
D_HID = 2048      # hidden size
K = D_IN + D_HID  # contraction dim = 4096
NCORES = 8
KT = K // 128     # 32 k-tiles
HC = D_HID // NCORES  # hidden units per core = 256
HTC = HC // 128   # hidden tiles per core = 2
MB = 512          # moving (batch) chunk
NMB = B // MB     # 8 batch chunks
KC = 4            # k-tiles per streamed comb chunk
F32 = mybir.dt.float32
F32R = mybir.dt.float32r
SIG = mybir.ActivationFunctionType.Sigmoid
TANH = mybir.ActivationFunctionType.Tanh

_CACHE = {}


def emit_prologue(nc, tc, ctx, tensors):
    """Load the resident tiles (W slice, bias)."""
    w, biasd = tensors["w"], tensors["bias"]
    res = ctx.enter_context(tc.tile_pool(name="res", bufs=1))
    w_sb = res.tile([128, 4, HTC, KT, 128], F32R)
    for g in range(4):
        for ht in range(HTC):
            nc.sync.dma_start(out=w_sb[:, g, ht], in_=w[g, ht])
    bias_sb = res.tile([128, 4 * HTC], F32)
    nc.sync.dma_start(out=bias_sb, in_=biasd[:, :])
    return w_sb, bias_sb


def emit_body(nc, tc, pools, tensors, resident):
    """One full LSTM-cell pass: 8 batch chunks x 2 hidden tiles."""
    combt, cprevt = tensors["combt"], tensors["cprevt"]
    h_out, c_out = tensors["h_out"], tensors["c_out"]
    w_sb, bias_sb = resident
    cpool, ps, ep = pools
    for mb in range(NMB):
        msl = slice(mb * MB, (mb + 1) * MB)
        accs = {}
        for ht in range(HTC):
            for g in range(4):
                accs[(ht, g)] = ps.tile(
                    [128, MB], F32, tag=f"acc{ht}{g}", name=f"acc{ht}{g}"
                )
        # k-outer accumulation so one streamed comb chunk feeds all 8 psums
        for kc in range(KT // KC):
            ksl = slice(kc * KC, (kc + 1) * KC)
            comb_c = cpool.tile([128, KC, MB], F32R, tag="comb")
            nc.sync.dma_start(out=comb_c, in_=combt[:, ksl, msl])
            for ht in range(HTC):
                for g in range(4):
                    acc = accs[(ht, g)]
                    for kk in range(KC):
                        k = kc * KC + kk
                        nc.tensor.matmul(
                            acc, lhsT=w_sb[:, g, ht, k, :], rhs=comb_c[:, kk, :],
                            start=(k == 0), stop=(k == KT - 1),
                        )
        for ht in range(HTC):
            gate_sb = []
            for g in range(4):
                out_g = ep.tile([128, MB], F32, tag=f"gate{g}", name=f"gate{g}")
                nc.scalar.activation(
                    out_g, accs[(ht, g)], TANH if g == 2 else SIG,
                    bias=bias_sb[:, g * HTC + ht:g * HTC + ht + 1],
                )
                gate_sb.append(out_g)
            f_s, i_s, ch, o_s = gate_sb
            cprev_c = ep.tile([128, MB], F32, tag="cprev")
            nc.sync.dma_start(out=cprev_c, in_=cprevt[:, ht, msl])
            t1 = ep.tile([128, MB], F32, tag="t1")
            nc.vector.tensor_mul(t1, f_s, cprev_c)
            t2 = ep.tile([128, MB], F32, tag="t2")
            nc.vector.tensor_mul(t2, i_s, ch)
            ct = ep.tile([128, MB], F32, tag="ct")
            nc.vector.tensor_add(ct, t1, t2)
            tct = ep.tile([128, MB], F32, tag="tct")
            nc.scalar.activation(tct, ct, TANH)
            htl = ep.tile([128, MB], F32, tag="htl")
            nc.vector.tensor_mul(htl, o_s, tct)
            nc.sync.dma_start(out=c_out[ht][:, msl], in_=ct)
            nc.sync.dma_start(out=h_out[ht][:, msl], in_=htl)


def make_pools(tc, ctx):
    cpool = ctx.enter_context(tc.tile_pool(name="cpool", bufs=3))
    ps = ctx.enter_context(tc.tile_pool(name="ps", bufs=1, space="PSUM"))
    ep = ctx.enter_context(tc.tile_pool(name="ep", bufs=2))
    return cpool, ps, ep


def _build():
    """Build + compile the per-core program (identical on all cores)."""
    nc = bacc.Bacc("TRN2", target_bir_lowering=False, debug=False)
    tensors = {
        "w": nc.declare_dram_parameter("w", [4, HTC, 128, KT, 128], F32R, isOutput=False),
        "combt": nc.declare_dram_parameter("combt", [128, KT, B], F32R, isOutput=False),
        "cprevt": nc.declare_dram_parameter("cprevt", [128, HTC, B], F32, isOutput=False),
        "bias": nc.declare_dram_parameter("bias", [128, 4 * HTC], F32, isOutput=False),
        "h_out": nc.declare_dram_parameter("h_out", [HTC, 128, B], F32, isOutput=True),
        "c_out": nc.declare_dram_parameter("c_out", [HTC, 128, B], F32, isOutput=True),
    }
    with ExitStack() as ctx:
        tc = ctx.enter_context(tile.TileContext(nc))
        resident = emit_prologue(nc, tc, ctx, tensors)
        pools = make_pools(tc, ctx)
        emit_body(nc, tc, pools, tensors, resident)
    nc.compile()
    return nc


def _get_nc():
    if "nc" not in _CACHE:
        _CACHE["nc"] = _build()
    return _CACHE["nc"]


def _prep_inputs(x_t, h_prev, c_prev, Wf, bf, Wi, bi, Wc, bc, Wo, bo):
    # Full W packed hidden-tile-major: [4, 16ht, 128p, KT, 128m]; each core
    # takes its 2 hidden tiles -> [4, 2, 128, KT, 128] (16 MiB, contiguous
    # per-partition DMA lines).
    w_pack = np.empty((4, D_HID // 128, 128, KT, 128), np.float32)
    for g, W in enumerate((Wf, Wi, Wc, Wo)):
        w_pack[g] = np.asarray(W).reshape(KT, 128, D_HID // 128, 128).transpose(2, 1, 0, 3)

    b_all = np.stack([np.asarray(b).reshape(D_HID // 128, 128) for b in (bf, bi, bc, bo)])

    combT = np.concatenate([np.asarray(x_t), np.asarray(h_prev)], axis=1).T
    combt_pack = np.ascontiguousarray(
        combT.reshape(KT, 128, B).transpose(1, 0, 2))        # [128, KT, B], shared
    cprevT = np.asarray(c_prev).T                            # [D_HID, B]

    in_maps = []
    for c in range(NCORES):
        hsl = slice(c * HTC, (c + 1) * HTC)  # hidden-tile slice
        w_c = np.ascontiguousarray(w_pack[:, hsl])           # [4, HTC, 128, KT, 128]
        b_c = np.ascontiguousarray(
            b_all[:, hsl].transpose(2, 0, 1).reshape(128, 4 * HTC))
        cprev_c = np.ascontiguousarray(
            cprevT[c * HC:(c + 1) * HC].reshape(HTC, 128, B).transpose(1, 0, 2))
        in_maps.append({
            "w": w_c,
            "bias": b_c,
            "combt": combt_pack,
            "cprevt": cprev_c,
        })
    return in_maps


def kernel(x_t, h_prev, c_prev, Wf, bf, Wi, bi, Wc, bc, Wo, bo):
    nc = _get_nc()
    in_maps = _prep_inputs(x_t, h_prev, c_prev, Wf, bf, Wi, bi, Wc, bc, Wo, bo)
    r = run_bass_kernel_spmd(nc, in_maps, core_ids=list(range(NCORES)))
    h_t = np.empty((D_HID, B), np.float32)
    c_t = np.empty((D_HID, B), np.float32)
    for c in range(NCORES):
        sl = slice(c * HC, (c + 1) * HC)
        h_t[sl] = r.results[c]["h_out"].reshape(HC, B)
        c_t[sl] = r.results[c]["c_out"].reshape(HC, B)
    return (np.ascontiguousarray(h_t.T), np.ascontiguousarray(c_t.T))
